# revision 20
# baseline (speedup 1.0000x reference)
"""Multi-head attention (qkv proj + softmax attention + out proj) on 8 trn2 cores.

Sharding: batch x query-half, ONE pairwise collective family. Core c handles
batch b=c//2 and query-half h=c%2 (1024 queries). The host passes only the
core's OWN 1024 tokens of x (contraction-major bf16); K and V projections are
computed for the local tokens only and the halves are exchanged between the
two cores of each batch with pairwise AllGathers (groups [[0,1],[2,3],...]),
so the duplicate half of the K/V projection work from the zero-collective
variant disappears from the PE stream. After the gather both cores see the
keys in rank order [even-half | odd-half]; key order is irrelevant to
softmax as long as K and V agree, so the SPMD program stays rank-symmetric.

Host-side marshaling: x and all weights pre-transposed to contraction-major
layout, pre-cast to bf16, packed into the exact tile layouts the device
wants; zero PE transposes on device.

Per-core device pipeline:
  1. QKV projection in bf16: QT/KT feature-major [d, t] so each 128-row tile
     holds a PAIR of heads; V token-major with interleaved ones columns
     (fused softmax-sum rows come out of the AV matmul free). K/V computed
     for local tokens, gathered pairwise through internal DRAM tiles.
  2. Attention per head-pair: QK^T into PSUM, softmax WITHOUT
     max-subtraction (scores ~N(0,1); fp32 exp is safe), exp on ScalarE
     straight out of PSUM, AV+sums fused, fast approximate reciprocal on DVE.
  3. Out-projection in bf16 + bias; y stored bf16 (host upcasts).
"""

import numpy as np

B, N, C = 4, 2048, 1024
H, D = 16, 64
P = 128
CG = C // P            # 8 contraction groups
TG = N // P            # 16 key-token chunks
TQ = N // 2            # 1024 queries per core
QB = 512               # query block (psum bank)
NB = TQ // QB          # 2
NPAIR = H // 2         # 8 head pairs
SCALE = 1.0 / np.sqrt(D).astype(np.float32)
PAIRS = [[0, 1], [2, 3], [4, 5], [6, 7]]

_CACHE = {}


def _build():
    import concourse.tile as tile
    from concourse import bacc, mybir

    f32 = mybir.dt.float32
    bf16 = mybir.dt.bfloat16
    nc = bacc.Bacc(
        "TRN2", target_bir_lowering=False, debug=False, num_devices=8
    )
    xT_h = nc.dram_tensor("xT", [P, CG, TQ], bf16, kind="ExternalInput").ap()
    wk_h = nc.dram_tensor("wk", [NPAIR, P, CG, P], bf16, kind="ExternalInput").ap()
    wq_h = nc.dram_tensor("wq", [NPAIR, P, CG, P], bf16, kind="ExternalInput").ap()
    wv_h = nc.dram_tensor("wv", [P, CG, C], bf16, kind="ExternalInput").ap()
    wo_h = nc.dram_tensor("wo", [P, CG, C], bf16, kind="ExternalInput").ap()
    bout_h = nc.dram_tensor("b_out", [C], f32, kind="ExternalInput").ap()
    y_h = nc.dram_tensor("y", [TQ, C], bf16, kind="ExternalOutput").ap()

    with tile.TileContext(nc) as tc:
        _emit(tc, xT_h, wk_h, wq_h, wv_h, wo_h, bout_h, y_h)
    nc.compile()
    return nc


def _emit(tc, xT_h, wk_h, wq_h, wv_h, wo_h, bout_h, y_h):
    from contextlib import ExitStack

    from concourse import mybir

    f32 = mybir.dt.float32
    bf16 = mybir.dt.bfloat16
    AF = mybir.ActivationFunctionType
    nc = tc.nc

    with ExitStack() as ctx:
        # ---------------- pools ----------------
        const = ctx.enter_context(tc.tile_pool(name="const", bufs=1))
        big = ctx.enter_context(tc.tile_pool(name="big", bufs=1))
        ktp = ctx.enter_context(tc.tile_pool(name="ktp", bufs=2))
        klp = ctx.enter_context(tc.tile_pool(name="klp", bufs=2))
        qtp = ctx.enter_context(tc.tile_pool(name="qtp", bufs=2))
        wkp = ctx.enter_context(tc.tile_pool(name="wkp", bufs=2))
        wqp = ctx.enter_context(tc.tile_pool(name="wqp", bufs=2))
        vlp = ctx.enter_context(tc.tile_pool(name="vlp", bufs=8))
        ptp = ctx.enter_context(tc.tile_pool(name="ptp", bufs=17))
        recp = ctx.enter_context(tc.tile_pool(name="recp", bufs=2))
        outp = ctx.enter_context(tc.tile_pool(name="outp", bufs=2))
        dram = ctx.enter_context(
            tc.tile_pool(name="dram", bufs=8, space="DRAM")
        )
        vdram = ctx.enter_context(
            tc.tile_pool(name="vdram", bufs=2, space="DRAM")
        )
        ps_pool = ctx.enter_context(
            tc.tile_pool(name="ps_pool", bufs=2, space="PSUM")
        )
        st_ps = ctx.enter_context(
            tc.tile_pool(name="st_ps", bufs=2, space="PSUM")
        )
        at_ps = ctx.enter_context(
            tc.tile_pool(name="at_ps", bufs=2, space="PSUM")
        )

        # ---------------- constants ----------------
        onesrow = const.tile([1, P], f32)
        nc.gpsimd.memset(onesrow, 1.0)
        bl = const.tile([1, C], f32)
        nc.gpsimd.dma_start(bl, bout_h.unsqueeze(0))

        # ---------------- persistent tensors ----------------
        # V layout per pair p: cols [192p,192p+64)=V_h2p, [192p+64,+128)=ones,
        # [192p+128,+192)=V_h2p+1. The AV stationary operand for head A is
        # cols [192p,192p+128) = [V_A | ones]; for head B it is cols
        # [192p+64,192p+192) = [ones | V_B]. Feature half fh occupies cols
        # [768fh, 768(fh+1)) since 768*(p//4)+192*(p%4) == 192p.
        xTt = big.tile([P, CG, TQ], bf16, name="xTt")
        xT = [xTt[:, g, :] for g in range(CG)]
        Vt = big.tile([P, TG, 192 * NPAIR], bf16, name="Vt")
        V = [Vt[:, i, :] for i in range(TG)]
        aoT = [big.tile([P, TQ], bf16, name=f"aoT{g}") for g in range(CG)]
        wv = big.tile([P, CG, C], bf16, name="wv")
        wo = big.tile([P, CG, C], bf16, name="wo")

        # -------- input DMAs, in startup-critical order --------
        # K proj chases the xT stream g by g, so land g0/g1 first and the
        # rest in pairs (few kicks, fine-grained deps where it matters).
        nc.sync.dma_start(xTt[:, 0, :], xT_h[:, 0, :])
        nc.sync.dma_start(xTt[:, 1, :], xT_h[:, 1, :])
        for g2 in range(2, CG, 2):
            nc.sync.dma_start(
                xTt[:, g2 : g2 + 2, :], xT_h[:, g2 : g2 + 2, :]
            )
        nc.sync.dma_start(wv[:, :, 0:QB], wv_h[:, :, 0:QB])
        nc.sync.dma_start(wv[:, :, QB:C], wv_h[:, :, QB:C])
        nc.sync.dma_start(wo, wo_h)

        # ---- deferred-emission machinery: projection work for pair p+1 is
        # emitted in small quanta INTO pair p's attention emission, so the
        # Tile scheduler gives it priorities that interleave it into the
        # PE-idle slots of the (Scalar-exp-paced) attention phase instead of
        # serializing it at the pair boundary.
        def k_proj_gen(p, out):
            # K pair p, local tokens only, then pairwise AllGather.
            wkt = wkp.tile([P, CG, P], bf16, tag="wk")
            nc.scalar.dma_start(wkt, wk_h[p])
            KT = ktp.tile([P, N], bf16, tag="KT")
            kloc = klp.tile([P, TQ], bf16, tag="kloc")
            kin = dram.tile([P, TQ], bf16, tag="kin")
            kout = dram.tile([2, P, TQ], bf16, tag="kout")
            yield
            for tb in range(2):
                ps = ps_pool.tile([P, QB], f32, tag="ps", name=f"pjk{tb}")
                for g in range(CG):
                    nc.tensor.matmul(
                        ps,
                        wkt[:, g, :],
                        xT[g][:, tb * QB : (tb + 1) * QB],
                        start=(g == 0),
                        stop=(g == CG - 1),
                    )
                    if g % 4 == 3:
                        yield
                nc.vector.tensor_copy(
                    kloc[:, tb * QB : (tb + 1) * QB], ps
                )
                yield
            nc.gpsimd.dma_start(kin[:], kloc)
            nc.gpsimd.collective_compute(
                "AllGather",
                mybir.AluOpType.bypass,
                replica_groups=PAIRS,
                ins=[kin.opt()],
                outs=[kout.opt()],
            )
            nc.gpsimd.dma_start(
                KT.rearrange("p (r t) -> p r t", r=2),
                kout[:].rearrange("r p t -> p r t"),
            )
            out["KT"] = KT

        def q_proj_gen(p, out):
            # Q pair p: queries are the local tokens. Scale 1/sqrt(D)
            # pre-folded into wq on the host.
            wqt = wqp.tile([P, CG, P], bf16, tag="wq")
            nc.scalar.dma_start(wqt, wq_h[p])
            QT = qtp.tile([P, TQ], bf16, tag="QT")
            yield
            pss = [
                ps_pool.tile([P, QB], f32, tag="ps", name=f"pjq{k}")
                for k in range(2)
            ]
            for g in range(CG):
                for k in range(2):
                    nc.tensor.matmul(
                        pss[k],
                        wqt[:, g, :],
                        xT[g][:, k * QB : (k + 1) * QB],
                        start=(g == 0),
                        stop=(g == CG - 1),
                    )
                yield
            for k in range(2):
                nc.vector.tensor_copy(QT[:, k * QB : (k + 1) * QB], pss[k])
            out["QT"] = QT

        def v_proj_gen(fh, i0, i1, st):
            # V feature half fh (heads 8fh..8fh+7, pairs 4fh..4fh+3), local
            # token chunks i0..i1-1; scatter into interleaved v_loc staging
            # tiles (with ones), then DMA out per-PAIR slices so each
            # attention pair can be gathered separately (384 KB apiece)
            # and the CC channel never front-loads.
            if "vin" not in st:
                st["vin"] = vdram.tile(
                    [4, CG, P, 192], bf16, tag="vin", name=f"vin{fh}"
                )
            vin = st["vin"]
            for i in range(i0, i1):
                vl = vlp.tile([P, 768], bf16, tag="vl")
                v3 = vl.rearrange("p (q e) -> p q e", e=64)
                nc.gpsimd.memset(v3[:, 1:12:3], 1.0)
                ps = ps_pool.tile([P, QB], f32, tag="ps")
                for g in range(CG):
                    nc.tensor.matmul(
                        ps,
                        xT[g][:, i * P : (i + 1) * P],
                        wv[:, g, fh * QB : (fh + 1) * QB],
                        start=(g == 0),
                        stop=(g == CG - 1),
                    )
                    if g % 4 == 3:
                        yield
                ps3 = ps.rearrange("p (k e) -> p k e", e=64)
                nc.vector.tensor_copy(v3[:, 0:12:3], ps3[:, 0::2])
                nc.vector.tensor_copy(v3[:, 2:12:3], ps3[:, 1::2])
                nc.gpsimd.dma_start(
                    vin[:, i].rearrange("q p e -> p q e"),
                    vl.rearrange("p (q e) -> p q e", e=192),
                )
                yield

        def v_gather_q(p):
            # Gather head-pair p's 192-col V slice (all 16 key chunks);
            # land both rank halves straight into the V tiles
            # (rank-symmetric: every core takes both halves from the
            # gathered buffer, keys end up in rank order
            # [even-half | odd-half] on both cores).
            fh, q = divmod(p, 4)
            vin = vst[fh]["vin"]
            vout = vdram.tile(
                [2, CG, P, 192], bf16, tag="vo", name=f"vo{p}", bufs=3
            )
            nc.gpsimd.collective_compute(
                "AllGather",
                mybir.AluOpType.bypass,
                replica_groups=PAIRS,
                ins=[vin[q].opt()],
                outs=[vout.opt()],
            )
            # Land each rank half with ONE strided DMA into the fused V
            # tile (16 tiny DMAs would serialize ~12us of queue kicks).
            for r in range(2):
                nc.gpsimd.dma_start(
                    Vt[:, r * CG : (r + 1) * CG, 192 * p : 192 * (p + 1)],
                    vout[r].rearrange("i p e -> p i e"),
                )

        def gathers_gen(ps_):
            yield
            for p_ in ps_:
                v_gather_q(p_)

        def v_chunk_gen(fh, i0, i1, st, tail_gathers=()):
            yield from v_proj_gen(fh, i0, i1, st)
            for p_ in tail_gathers:
                v_gather_q(p_)

        def bias_gen(out):
            bias = big.tile([P, C], f32, name="bias")
            yield
            for hh in range(2):
                ps = ps_pool.tile([P, QB], f32, tag="ps")
                nc.tensor.matmul(
                    ps, onesrow, bl[0:1, hh * QB : (hh + 1) * QB]
                )
                nc.scalar.copy(bias[:, hh * QB : (hh + 1) * QB], ps)
                yield
            out["bias"] = bias

        def out_proj_gen(i0, i1, ow):
            for i in range(i0, i1):
                ob = outp.tile([P, C], bf16, tag="ob")
                for oh in range(2):
                    ps = ps_pool.tile([P, QB], f32, tag="ps")
                    for g in range(CG):
                        nc.tensor.matmul(
                            ps,
                            aoT[g][:, i * P : (i + 1) * P],
                            wo[:, g, oh * QB : (oh + 1) * QB],
                            start=(g == 0),
                            stop=(g == CG - 1),
                        )
                        if g % 4 == 3:
                            yield
                    nc.vector.tensor_add(
                        ob[:, oh * QB : (oh + 1) * QB],
                        ps,
                        ow["bias"][:, oh * QB : (oh + 1) * QB],
                    )
                nc.sync.dma_start(y_h[i * P : (i + 1) * P, :], ob)
                yield

        pending = []

        def pump(n=1):
            for _ in range(n):
                while pending:
                    try:
                        next(pending[0])
                        break
                    except StopIteration:
                        pending.pop(0)
                else:
                    return

        def drain():
            while pending:
                pump()

        # ---- lagged AV: each block's AV + normalize is emitted chunk-by-
        # chunk INSIDE the next block's QK loop, so the Scalar exp chain
        # never stalls behind an AV wall at block/pair boundaries.
        def emit_av_chunk(p, st, ptab, j, first, last):
            nc.tensor.matmul(
                st["ata"],
                V[j][:, 192 * p : 192 * p + 128],
                ptab[:, 0:QB],
                start=first,
                stop=last,
            )
            nc.tensor.matmul(
                st["atb"],
                V[j][:, 192 * p + 64 : 192 * p + 192],
                ptab[:, QB : 2 * QB],
                start=first,
                stop=last,
            )

        def emit_normalize(p, tb, st):
            # out = at * (1/sum). reciprocal_approx_fast (custom-DVE)
            # requires base-partition-0 APs, so stage sumsA down to a
            # base-0 tile; mixed PSUM+SBUF operands may use different base
            # partitions, so the muls read the reciprocal tiles directly.
            ata, atb = st["ata"], st["atb"]
            combA = recp.tile([64, QB], f32, tag="combA", bufs=1)
            nc.vector.tensor_copy(combA, ata[64:128, :])
            rtA = recp.tile([64, QB], f32, tag="rtA", bufs=1)
            nc.vector.reciprocal_approx_fast(rtA, combA)
            rtB = recp.tile([64, QB], f32, tag="rtB", bufs=1)
            nc.vector.reciprocal_approx_fast(rtB, atb[0:64, :])
            ao = aoT[p][:, tb * QB : (tb + 1) * QB]
            nc.vector.tensor_mul(ao[0:64, :], ata[0:64, :], rtA)
            nc.vector.tensor_mul(ao[64:128, :], atb[64:128, :], rtB)

        def make_av_steps(p, tb, pts):
            st = {}

            def step(idx):
                j = idx

                def run():
                    if idx == 0:
                        st["ata"] = at_ps.tile([P, QB], f32, tag="at", name="ata")
                        st["atb"] = at_ps.tile([P, QB], f32, tag="at", name="atb")
                    emit_av_chunk(p, st, pts[j], j, idx == 0, idx == TG - 1)
                    if idx == TG - 1:
                        emit_normalize(p, tb, st)

                return run

            return [step(idx) for idx in range(TG)]

        def emit_qk_chunk(KT, qa, qb, j, pts):
            # Per key chunk j, ONE 2-bank psum tile holds head A scores in
            # cols 0:QB and head B in QB:2QB, covered by ONE exp. Both QK
            # matmuls then wait on the same semaphore, co-dispatch, and run
            # concurrently in disjoint PE row groups (tile_position (0,0)
            # / (64,0) auto-derived from the 64-partition operands).
            stab = st_ps.tile([P, 2 * QB], f32, tag="st", name="stab")
            nc.tensor.matmul(stab[:, 0:QB], KT[0:64, j * P : (j + 1) * P], qa)
            nc.tensor.matmul(
                stab[:, QB : 2 * QB], KT[64:128, j * P : (j + 1) * P], qb
            )
            ptab = ptp.tile([P, 2 * QB], bf16, tag="pt", name="ptab")
            nc.scalar.activation(ptab, stab, AF.Exp)
            pts.append(ptab)

        # ------------- prologue: K0 local + gather kicked ASAP, then ALL of
        # V fh0 local proj (fills the PE while the K0 gather is in flight),
        # pair-0's V slice gather, then Q0. Each later pair's 384 KB V
        # slice gather is emitted one pair ahead, interleaving with the K
        # gathers on the CC channel so no single deadline is tight. --------
        kq = {}
        vst = [{}, {}]
        for _ in k_proj_gen(0, kq):
            pass
        for _ in v_chunk_gen(0, 0, CG, vst[0], tail_gathers=(0,)):
            pass
        for _ in q_proj_gen(0, kq):
            pass
        ow = {}
        lag = []
        pending.append(bias_gen(ow))

        # ---------------- per-pair: K proj, Q proj, attention --------------
        for p in range(NPAIR):
            KT, QT = kq["KT"], kq["QT"]
            kq = {}
            if p + 1 < NPAIR:
                # next pair's K/Q proj is urgent: queue front (K first so
                # its gather gets kicked as early as possible)
                pending.insert(0, q_proj_gen(p + 1, kq))
                pending.insert(0, k_proj_gen(p + 1, kq))
            if p == 0:
                pending.append(gathers_gen([1]))
            elif p == 1:
                pending.append(gathers_gen([2]))
                pending.append(v_chunk_gen(1, 0, 4, vst[1]))
            elif p == 2:
                pending.append(gathers_gen([3]))
                pending.append(v_chunk_gen(1, 4, CG, vst[1], tail_gathers=(4,)))
            elif p in (3, 4, 5):
                # fh1 slices: FIFO order guarantees these emit after the
                # fh1 proj + its vin DMAs have fully drained.
                pending.append(gathers_gen([p + 2]))

            if p < NPAIR - 1:
                for tb in range(NB):
                    qa = QT[0:64, tb * QB : (tb + 1) * QB]
                    qb = QT[64:128, tb * QB : (tb + 1) * QB]
                    pts = []
                    for j in range(TG):
                        if lag:
                            lag.pop(0)()
                        emit_qk_chunk(KT, qa, qb, j, pts)
                        pump(2)
                    assert not lag
                    lag = make_av_steps(p, tb, pts)
                # make sure the next pair's KT/QT is fully emitted
                while "QT" not in kq:
                    pump()
            else:
                # last pair: inline AV so out-proj can chase each block
                for tb in range(NB):
                    qa = QT[0:64, tb * QB : (tb + 1) * QB]
                    qb = QT[64:128, tb * QB : (tb + 1) * QB]
                    pts = []
                    for j in range(TG):
                        if lag:
                            lag.pop(0)()
                        emit_qk_chunk(KT, qa, qb, j, pts)
                        pump(2)
                    st = {
                        "ata": at_ps.tile([P, QB], f32, tag="at", name="ata"),
                        "atb": at_ps.tile([P, QB], f32, tag="at", name="atb"),
                    }
                    for j in range(TG):
                        emit_av_chunk(p, st, pts[j], j, j == 0, j == TG - 1)
                        if j % 4 == 3:
                            pump()
                    emit_normalize(p, tb, st)
                    # all pairs done for this query block: out-project it,
                    # overlapping the last pair's remaining attention work
                    drain()
                    pending.append(out_proj_gen(tb * 4, (tb + 1) * 4, ow))
                    if tb == NB - 1:
                        drain()


def _run(in_maps, trace=False):
    from concourse.bass_utils import run_bass_kernel_spmd

    if "nc" not in _CACHE:
        _CACHE["nc"] = _build()
    nc = _CACHE["nc"]
    return run_bass_kernel_spmd(
        nc, in_maps, core_ids=list(range(8)), trace=trace
    )


def _make_in_maps(x, w_qkv, w_out, b_out):
    import ml_dtypes

    bf16 = ml_dtypes.bfloat16
    x = np.asarray(x, dtype=np.float32)
    w_qkv = np.asarray(w_qkv, dtype=np.float32)
    w_out = np.asarray(w_out, dtype=np.float32)
    b_out = np.ascontiguousarray(np.asarray(b_out, dtype=np.float32))

    # Device-layout weight packs (shared by all 8 cores).
    # wk/wq: [pair, part(=cin%128), g(=cin//128), col(=feat within pair)]
    def kq_pack(wrows):
        a = wrows.T.reshape(CG, P, NPAIR, P)          # [g, r, pair, c]
        return np.ascontiguousarray(
            a.transpose(2, 1, 0, 3).astype(bf16)      # [pair, r, g, c]
        )

    wq = kq_pack(w_qkv[0:C] * SCALE)
    wk = kq_pack(w_qkv[C : 2 * C])
    # wv/wo: [part(=cin%128), g(=cin//128), feat]
    def vo_pack(wrows):
        a = wrows.T.reshape(CG, P, C)                 # [g, r, f]
        return np.ascontiguousarray(a.transpose(1, 0, 2).astype(bf16))

    wv = vo_pack(w_qkv[2 * C : 3 * C])
    wo = vo_pack(w_out)

    in_maps = []
    for c in range(8):
        b, h = divmod(c, 2)
        xloc = x[b][h * TQ : (h + 1) * TQ]            # local tokens only
        xT = np.ascontiguousarray(
            xloc.T.astype(bf16).reshape(CG, P, TQ).transpose(1, 0, 2)
        )
        in_maps.append(
            {"xT": xT, "wk": wk, "wq": wq, "wv": wv, "wo": wo,
             "b_out": b_out}
        )
    return in_maps


def _gather(results):
    y = np.empty((B, N, C), dtype=np.float32)
    for c in range(8):
        b, h = divmod(c, 2)
        y[b, h * TQ : (h + 1) * TQ, :] = results[c]["y"].astype(np.float32)
    return y


def kernel(x, w_qkv, w_out, b_out):
    res = _run(_make_in_maps(x, w_qkv, w_out, b_out), trace=False)
    return _gather(res.results)
